# revision 6
# baseline (speedup 1.0000x reference)
"""Trainium2 Bass kernel for nn_AttentionOutput (complex causal leaky-relu attention).

Reference (B=4, N=4096, F=64), per batch:
    sr = (Qr@Kr^T - Qi@Ki^T)/sqrt(N); si = (Qr@Ki^T + Qi@Kr^T)/sqrt(N)
    wr = tril * leaky_relu(sr);        wi = tril * leaky_relu(si)
    out_r = (wr@Vr)@W_att^T + b;       out_i = (wi@Vi)@W_att^T + b

Distribution: 2 cores per batch.  Core parity h processes j-blocks J === h
(mod 2) for ALL 4096 query rows; causal work is then identical across cores
(slot I needs 2I+2 j-blocks), so a single SPMD program serves all 8 cores and
the host sums the two partial outputs per batch.

Host-side layout prep removes every on-device transpose:
  - scores contract over p = f*2+c (128 partitions, ONE matmul per component):
    sr = Qmodr . K^T where Qmodr = Q with odd columns negated, and
    si = Qmodi . K^T where Qmodi = Q with column pairs swapped; K stays plain.
    Both Q variants are fed pre-transposed [128, N].
  - V' = (1/64) V @ W_att^T folds the score scale and the output projection
    into the attention-value matmul (leaky_relu is positively homogeneous).
  - output is stored transposed ([128, N]: y_r^T on rows 0:64, y_i^T on
    64:128); the host untransposes, interleaves, adds bias, sums parities.

v2 structure (vs v1 baseline at 131us):
  - scores for r and i go into ONE 3D PSUM tile [128, 2, 512] (2 banks);
    the relu drain covers both components in ONE DVE/ACT op (fixed-cost
    amortized), alternating VectorE / ScalarE by a 1:2 pattern.
  - y_r^T and y_i^T share ONE [128, 512] PSUM bank: value matmuls are
    col-group packed (r -> partitions 0:64 via stationary cols 0:64,
    i -> partitions 64:128).  The PE runs such M=64 pairs concurrently in
    disjoint column groups, roughly halving value-matmul time.
  - diagonal blocks: the second diag j-block of each slot is fully masked
    for i-offsets < 256 on both core parities, so score matmuls, drains and
    value matmuls subrange to [256:512] there.
  - PSUM: 3 score bufs (6 banks) + 2 y bufs (2 banks) = 8 banks.

leaky_relu lowering (RELU_CORR): leaky(s) = 0.99*relu(s) + 0.01*s.  For
causally-full j-blocks the 0.01*s term telescopes into a per-slot constant
matmul: mcorr = 0.01*sum_J kp_J @ V'_J, precomputed on the host and
accumulated into the y PSUM bank.  Diagonal tiles compute u = mask*s
(VectorE, drains) and w = relu(u), feeding matmuls against 0.01*V' and
0.99*V'.

NOTE: ACT Lrelu reading PSUM hangs TRN2 (empirically) — never emit it.
"""

import numpy as np

import concourse.bacc as bacc
import concourse.tile as tile
from concourse import mybir
from concourse.bass_utils import run_bass_kernel_spmd

B, N, F = 4, 4096, 64
P = 128             # = 2*F: score contraction width / partition count
JB = 128            # j-block width
IBW = 512           # i-block (slot) width
NSLOT = N // IBW    # 8 slots
NJPAR = N // JB // 2  # 16 parity j-blocks per core
NEG = 0.01
SCALE = 1.0 / 64.0  # 1/sqrt(N)
NCORES = 8
DIAG1_OFF = 256     # both parities: 2nd diag block dead for i-offset < 256

_DT = mybir.dt.float32
MM_BF16 = True      # bf16 matmul inputs: half the DMA bytes, 4x fp32 PE rate
SIM_SAFE_LRELU = True  # kept for test.py compat (unused)
_CACHE: dict = {}


def _build_nc():
    nc = bacc.Bacc("TRN2", target_bir_lowering=False, num_devices=NCORES)
    dt = _DT
    mdt = mybir.dt.bfloat16 if MM_BF16 else _DT  # matmul input dtype
    qrT = nc.dram_tensor("qrT", [P, N], mdt, kind="ExternalInput")
    qiT = nc.dram_tensor("qiT", [P, N], mdt, kind="ExternalInput")
    kp = nc.dram_tensor("kp", [P, NJPAR * JB], mdt, kind="ExternalInput")
    # va = 0.99 * V' (relu term), vb = 0.01 * V' (raw term, diagonal only)
    var_ = nc.dram_tensor("var", [P, NJPAR * F], mdt, kind="ExternalInput")
    vai = nc.dram_tensor("vai", [P, NJPAR * F], mdt, kind="ExternalInput")
    vbr = nc.dram_tensor("vbr", [P, NJPAR * F], mdt, kind="ExternalInput")
    vbi = nc.dram_tensor("vbi", [P, NJPAR * F], mdt, kind="ExternalInput")
    # per-slot correction weights: 0.01 * sum_{full J} kp_J @ V'_J  [P, 64]
    mcr = nc.dram_tensor("mcr", [P, NSLOT * F], mdt, kind="ExternalInput")
    mci = nc.dram_tensor("mci", [P, NSLOT * F], mdt, kind="ExternalInput")
    # masks duplicated for both components: [2, JB, 2, IBW]
    dmask = nc.dram_tensor("dmask", [2, JB, 2, IBW], mdt, kind="ExternalInput")
    out = nc.dram_tensor("out", [P, N], dt, kind="ExternalOutput")

    relu = mybir.ActivationFunctionType.Relu
    mul_op = mybir.AluOpType.mult

    with tile.TileContext(nc) as tc:
        with (
            tc.tile_pool(name="res", bufs=1) as res,
            tc.tile_pool(name="wp", bufs=3) as wp,
            tc.tile_pool(name="osb", bufs=2) as osb,
            tc.tile_pool(name="spsum", bufs=3, space="PSUM") as spsum,
            tc.tile_pool(name="ypsum", bufs=2, space="PSUM") as ypsum,
        ):
            # ---- input staging; order so slot 0's operands land first ----
            sb_qr = res.tile([P, N], mdt, tag="qr")
            sb_qi = res.tile([P, N], mdt, tag="qi")
            sb_k = res.tile([P, NJPAR * JB], mdt, tag="k")
            sb_m0 = res.tile([JB, 2, IBW], mdt, tag="m0")
            sb_m1 = res.tile([JB, 2, IBW], mdt, tag="m1")
            sb_var = res.tile([P, NJPAR * F], mdt, tag="var")
            sb_vai = res.tile([P, NJPAR * F], mdt, tag="vai")
            sb_vbr = res.tile([P, NJPAR * F], mdt, tag="vbr")
            sb_vbi = res.tile([P, NJPAR * F], mdt, tag="vbi")
            sb_mcr = res.tile([P, NSLOT * F], mdt, tag="mcr")
            sb_mci = res.tile([P, NSLOT * F], mdt, tag="mci")

            # Critical-path first chunks: split into partition stripes across
            # three engine DMA queues so slot 0's operands land fast.
            engs = (nc.sync, nc.gpsimd, nc.scalar)
            for e in range(3):
                psl = slice(e * 43, 128 if e == 2 else (e + 1) * 43)
                eng = engs[e]
                eng.dma_start(out=sb_qr[psl, 0:512], in_=qrT[psl, 0:512])
                eng.dma_start(out=sb_k[psl, 0:512], in_=kp[psl, 0:512])
                eng.dma_start(out=sb_qi[psl, 0:512], in_=qiT[psl, 0:512])
            nc.sync.dma_start(out=sb_m0, in_=dmask[0])
            nc.gpsimd.dma_start(out=sb_m1, in_=dmask[1])
            nc.scalar.dma_start(out=sb_vbr[:, 0:512], in_=vbr[:, 0:512])
            nc.scalar.dma_start(out=sb_vbi[:, 0:512], in_=vbi[:, 0:512])
            nc.sync.dma_start(out=sb_var[:, 0:512], in_=var_[:, 0:512])
            nc.gpsimd.dma_start(out=sb_vai[:, 0:512], in_=vai[:, 0:512])
            # remaining chunks ordered by first use (slot s needs q chunk s,
            # k chunk s//2, v chunk 1 from slot 4, mcorr from slot 1's end)
            nc.sync.dma_start(out=sb_qr[:, 512:1024], in_=qrT[:, 512:1024])
            nc.gpsimd.dma_start(out=sb_qi[:, 512:1024], in_=qiT[:, 512:1024])
            nc.sync.dma_start(out=sb_mcr, in_=mcr[:])
            nc.gpsimd.dma_start(out=sb_mci, in_=mci[:])
            nc.sync.dma_start(out=sb_k[:, 512:1024], in_=kp[:, 512:1024])
            for c in range(2, 8):
                sl = slice(c * 512, (c + 1) * 512)
                nc.sync.dma_start(out=sb_qr[:, sl], in_=qrT[:, sl])
                nc.gpsimd.dma_start(out=sb_qi[:, sl], in_=qiT[:, sl])
                if c < 4:
                    nc.sync.dma_start(out=sb_k[:, sl], in_=kp[:, sl])
                if c == 4:
                    nc.gpsimd.dma_start(out=sb_vbr[:, 512:1024],
                                        in_=vbr[:, 512:1024])
                    nc.sync.dma_start(out=sb_vbi[:, 512:1024],
                                      in_=vbi[:, 512:1024])
                    nc.gpsimd.dma_start(out=sb_var[:, 512:1024],
                                        in_=var_[:, 512:1024])
                    nc.sync.dma_start(out=sb_vai[:, 512:1024],
                                      in_=vai[:, 512:1024])
            sb_masks = (sb_m0, sb_m1)

            # Warm the PE (HAM clock gate) with dummy matmuls while the first
            # DMAs are in flight: ~16 small MMs keep the array busy so real
            # matmuls start at 2.4 GHz.
            warm_sb = res.tile([P, F], mdt, tag="warm")
            nc.vector.memset(warm_sb[:], 0.0)
            warm_y = ypsum.tile([P, IBW], dt, tag="y")
            for _ in range(16):
                nc.tensor.matmul(warm_y[0:64, 0:64], warm_sb[:], warm_sb[:],
                                 start=True, stop=True)

            drain_ctr = 0  # full-tile relu drains: cycle V,S,S
            for s in range(NSLOT):
                cnt = 2 * s + 2
                isl = slice(s * IBW, (s + 1) * IBW)
                y = ypsum.tile([P, IBW], dt, tag="y")
                for p in range(cnt):
                    ksl = slice(p * JB, (p + 1) * JB)
                    vsl = slice(p * F, (p + 1) * F)
                    diag = p >= cnt - 2
                    k = p - (cnt - 2)
                    o = DIAG1_OFF if (diag and k == 1) else 0
                    qsl = slice(s * IBW + o, (s + 1) * IBW)
                    first = p == 0

                    s2 = spsum.tile([P, 2, IBW], dt, tag="s")
                    nc.tensor.matmul(s2[:, 0:1, o:], sb_k[:, ksl],
                                     sb_qr[:, qsl], start=True, stop=True)
                    nc.tensor.matmul(s2[:, 1:2, o:], sb_k[:, ksl],
                                     sb_qi[:, qsl], start=True, stop=True)

                    if not diag:
                        # full block: w = relu(s); 0.01*s is in mcorr
                        w = wp.tile([P, 2, IBW], mdt, tag="w")
                        if drain_ctr % 3 == 0:
                            nc.vector.tensor_scalar_max(w[:], s2[:], 0.0)
                        else:
                            nc.scalar.activation(w[:], s2[:], relu)
                        drain_ctr += 1
                        nc.tensor.matmul(y[0:64, :], sb_var[:, vsl],
                                         w[:, 0:1, :], start=first, stop=False)
                        nc.tensor.matmul(y[64:128, :], sb_vai[:, vsl],
                                         w[:, 1:2, :], start=first, stop=False)
                    else:
                        # diagonal block: u = mask*s (drains), w = relu(u)
                        mk = sb_masks[k]
                        u = wp.tile([P, 2, IBW], mdt, tag="u")
                        nc.vector.tensor_tensor(out=u[:, :, o:], in0=s2[:, :, o:],
                                                in1=mk[:, :, o:], op=mul_op)
                        w = wp.tile([P, 2, IBW], mdt, tag="w")
                        nc.vector.tensor_scalar_max(w[:, :, o:], u[:, :, o:], 0.0)
                        nc.tensor.matmul(y[0:64, o:], sb_vbr[:, vsl],
                                         u[:, 0:1, o:], start=first, stop=False)
                        nc.tensor.matmul(y[64:128, o:], sb_vbi[:, vsl],
                                         u[:, 1:2, o:], start=first, stop=False)
                        last = s == 0 and p == cnt - 1
                        nc.tensor.matmul(y[0:64, o:], sb_var[:, vsl],
                                         w[:, 0:1, o:], start=False, stop=last)
                        nc.tensor.matmul(y[64:128, o:], sb_vai[:, vsl],
                                         w[:, 1:2, o:], start=False, stop=last)
                # correction matmul: y += (0.01 * sum_full kp_J @ V'_J)^T @ q
                if s > 0:
                    msl = slice(s * F, (s + 1) * F)
                    nc.tensor.matmul(y[0:64, :], sb_mcr[:, msl],
                                     sb_qr[:, isl], start=False, stop=True)
                    nc.tensor.matmul(y[64:128, :], sb_mci[:, msl],
                                     sb_qi[:, isl], start=False, stop=True)
                # tail: accumulator to SBUF (alternate V/S), then DMA out
                y_sb = osb.tile([P, IBW], dt, tag="ysb")
                if s % 2 == 0:
                    nc.vector.tensor_copy(y_sb[:], y[:])
                else:
                    nc.scalar.copy(y_sb[:], y[:])
                nc.sync.dma_start(out=out[:, isl], in_=y_sb[:])
    nc.compile()
    return nc


def _prep_inputs(Q, K, V, W_att, b_att):
    """Host-side re-layout: per-core in_maps for run_bass_kernel_spmd."""
    Q = np.asarray(Q, dtype=np.float32)
    K = np.asarray(K, dtype=np.float32)
    V = np.asarray(V, dtype=np.float32)
    W_att = np.asarray(W_att, dtype=np.float32)

    Qf = Q.reshape(B, N, P)          # [b, i, f*2+c]
    Kf = K.reshape(B, N, P)
    Vpr = SCALE * (V[..., 0] @ W_att.T)   # [B, N, F]
    Vpi = SCALE * (V[..., 1] @ W_att.T)

    # causal masks for a slot's last two parity j-blocks, per core parity h:
    # diagonal sub-block d = 2k+h of the slot's group of 4; duplicated along
    # a component axis -> [2, JB, 2, IBW]
    jj = np.arange(JB)[:, None]
    ii = np.arange(IBW)[None, :]
    masks = {}
    for h in (0, 1):
        mk = np.stack([(ii >= jj + JB * (2 * k + h)).astype(np.float32)
                       for k in range(2)])            # [2, JB, IBW]
        masks[h] = np.repeat(mk[:, :, None, :], 2, axis=2)  # [2, JB, 2, IBW]

    if MM_BF16:
        import ml_dtypes
        cvt = lambda a: np.ascontiguousarray(a).astype(ml_dtypes.bfloat16)
    else:
        cvt = lambda a: np.ascontiguousarray(a, dtype=np.float32)

    in_maps = []
    for c in range(NCORES):
        b, h = divmod(c, 2)
        Qmodr = Qf[b].copy()
        Qmodr[:, 1::2] *= -1.0
        Qmodi = np.empty_like(Qf[b])
        Qmodi[:, 0::2] = Qf[b][:, 1::2]
        Qmodi[:, 1::2] = Qf[b][:, 0::2]
        # parity-packed K: [P, NJPAR*JB], position pp holds block J = 2*pp+h
        kp3 = Kf[b].reshape(N // JB, JB, P)[h::2]          # [16, j, p]
        kp = kp3.transpose(2, 0, 1).reshape(P, -1)         # [p, pp*JB+j]
        vr3 = Vpr[b].reshape(N // JB, JB, F)[h::2]         # [16, j, f]
        vi3 = Vpi[b].reshape(N // JB, JB, F)[h::2]
        vpr = vr3.transpose(1, 0, 2).reshape(JB, -1)       # [j, pp*F+f]
        vpi = vi3.transpose(1, 0, 2).reshape(JB, -1)
        # per-slot correction: 0.01 * sum over FULL blocks (pos < cnt-2 = 2s)
        prod_r = np.einsum('bjp,bjf->bpf', kp3, vr3)       # [16, p, f]
        prod_i = np.einsum('bjp,bjf->bpf', kp3, vi3)
        pre_r = np.concatenate(
            [np.zeros((1, P, F), np.float32), np.cumsum(prod_r, axis=0)])
        pre_i = np.concatenate(
            [np.zeros((1, P, F), np.float32), np.cumsum(prod_i, axis=0)])
        mcr = np.concatenate([NEG * pre_r[2 * s] for s in range(NSLOT)], axis=1)
        mci = np.concatenate([NEG * pre_i[2 * s] for s in range(NSLOT)], axis=1)
        in_maps.append({
            "qrT": cvt(Qmodr.T),
            "qiT": cvt(Qmodi.T),
            "kp": cvt(kp),
            "var": cvt((1.0 - NEG) * vpr),
            "vai": cvt((1.0 - NEG) * vpi),
            "vbr": cvt(NEG * vpr),
            "vbi": cvt(NEG * vpi),
            "mcr": cvt(mcr),
            "mci": cvt(mci),
            "dmask": cvt(masks[h]),
        })
    return in_maps


def _gather(results, b_att):
    b_att = np.asarray(b_att, dtype=np.float32)
    out = np.empty((B, N, F, 2), dtype=np.float32)
    for b in range(B):
        y = results[2 * b]["out"] + results[2 * b + 1]["out"]  # [128, N]
        out[b, :, :, 0] = y[0:64].T + b_att[None, :]
        out[b, :, :, 1] = y[64:128].T + b_att[None, :]
    return out


def kernel(Q, K, V, W_att, b_att):
    if "nc" not in _CACHE:
        _CACHE["nc"] = _build_nc()
    nc = _CACHE["nc"]
    in_maps = _prep_inputs(Q, K, V, W_att, b_att)
    res = run_bass_kernel_spmd(nc, in_maps, core_ids=list(range(NCORES)))
    return _gather(res.results, b_att)


# revision 10
# speedup vs baseline: 1.0975x; 1.0975x over previous
"""Trainium2 Bass kernel for nn_AttentionOutput (complex causal leaky-relu attention).

Reference (B=4, N=4096, F=64), per batch:
    sr = (Qr@Kr^T - Qi@Ki^T)/sqrt(N); si = (Qr@Ki^T + Qi@Kr^T)/sqrt(N)
    wr = tril * leaky_relu(sr);        wi = tril * leaky_relu(si)
    out_r = (wr@Vr)@W_att^T + b;       out_i = (wi@Vi)@W_att^T + b

Distribution: 2 cores per batch.  Core parity h processes j-blocks J === h
(mod 2) for ALL 4096 query rows; causal work is then identical across cores
(slot I needs 2I+2 j-blocks), so a single SPMD program serves all 8 cores and
the host sums the two partial outputs per batch.

Host-side layout prep removes every on-device transpose:
  - scores contract over p = f*2+c (128 partitions, ONE matmul per component):
    sr = Qmodr . K^T where Qmodr = Q with odd columns negated, and
    si = Qmodi . K^T where Qmodi = Q with column pairs swapped; K stays plain.
    Both Q variants are fed pre-transposed [128, N].
  - V' = (1/64) V @ W_att^T folds the score scale and the output projection
    into the attention-value matmul (leaky_relu is positively homogeneous).
  - output is stored transposed ([128, N]: y_r^T on rows 0:64, y_i^T on
    64:128); the host untransposes, interleaves, adds bias, sums parities.

v2 structure (vs v1 baseline at 131us):
  - scores for r and i go into ONE 3D PSUM tile [128, 2, 512] (2 banks);
    the relu drain covers both components in ONE DVE/ACT op (fixed-cost
    amortized), alternating VectorE / ScalarE by a 1:2 pattern.
  - y_r^T and y_i^T share ONE [128, 512] PSUM bank: value matmuls are
    col-group packed (r -> partitions 0:64 via stationary cols 0:64,
    i -> partitions 64:128).  The PE runs such M=64 pairs concurrently in
    disjoint column groups, roughly halving value-matmul time.
  - diagonal blocks: the second diag j-block of each slot is fully masked
    for i-offsets < 256 on both core parities, so score matmuls, drains and
    value matmuls subrange to [256:512] there.
  - PSUM: 3 score bufs (6 banks) + 2 y bufs (2 banks) = 8 banks.

leaky_relu lowering (RELU_CORR): leaky(s) = 0.99*relu(s) + 0.01*s.  For
causally-full j-blocks the 0.01*s term telescopes into a per-slot constant
matmul: mcorr = 0.01*sum_J kp_J @ V'_J, precomputed on the host and
accumulated into the y PSUM bank.  Diagonal tiles compute u = mask*s
(VectorE, drains) and w = relu(u), feeding matmuls against 0.01*V' and
0.99*V'.

NOTE: ACT Lrelu reading PSUM hangs TRN2 (empirically) — never emit it.
"""

import numpy as np

import concourse.bacc as bacc
import concourse.tile as tile
from concourse import mybir
from concourse.bass_utils import run_bass_kernel_spmd

B, N, F = 4, 4096, 64
P = 128             # = 2*F: score contraction width / partition count
JB = 128            # j-block width
IBW = 512           # i-block (slot) width
NSLOT = N // IBW    # 8 slots
NJPAR = N // JB // 2  # 16 parity j-blocks per core
NEG = 0.01
SCALE = 1.0 / 64.0  # 1/sqrt(N)
NCORES = 8
DIAG1_OFF = 256     # both parities: 2nd diag block dead for i-offset < 256

_DT = mybir.dt.float32
MM_BF16 = True      # bf16 matmul inputs: half the DMA bytes, 4x fp32 PE rate
SIM_SAFE_LRELU = True  # kept for test.py compat (unused)
_CACHE: dict = {}


def _build_nc():
    nc = bacc.Bacc("TRN2", target_bir_lowering=False, num_devices=NCORES)
    dt = _DT
    mdt = mybir.dt.bfloat16 if MM_BF16 else _DT  # matmul input dtype
    qrT = nc.dram_tensor("qrT", [P, N], mdt, kind="ExternalInput")
    qiT = nc.dram_tensor("qiT", [P, N], mdt, kind="ExternalInput")
    kp = nc.dram_tensor("kp", [P, NJPAR * JB], mdt, kind="ExternalInput")
    # va = 0.99 * V' (relu term), vb = 0.01 * V' (raw term, diagonal only)
    var_ = nc.dram_tensor("var", [P, NJPAR * F], mdt, kind="ExternalInput")
    vai = nc.dram_tensor("vai", [P, NJPAR * F], mdt, kind="ExternalInput")
    vbr = nc.dram_tensor("vbr", [P, NJPAR * F], mdt, kind="ExternalInput")
    vbi = nc.dram_tensor("vbi", [P, NJPAR * F], mdt, kind="ExternalInput")
    # per-slot correction weights: 0.01 * sum_{full J} kp_J @ V'_J  [P, 64]
    mcr = nc.dram_tensor("mcr", [P, NSLOT * F], mdt, kind="ExternalInput")
    mci = nc.dram_tensor("mci", [P, NSLOT * F], mdt, kind="ExternalInput")
    # masks duplicated for both components: [2, JB, 2, IBW]
    dmask = nc.dram_tensor("dmask", [2, JB, 2, IBW], mdt, kind="ExternalInput")
    out = nc.dram_tensor("out", [P, N], mdt, kind="ExternalOutput")

    relu = mybir.ActivationFunctionType.Relu
    mul_op = mybir.AluOpType.mult
    max_op = mybir.AluOpType.max
    add_op = mybir.AluOpType.add

    with tile.TileContext(nc) as tc:
        with (
            tc.tile_pool(name="res", bufs=1) as res,
            tc.tile_pool(name="wp", bufs=3) as wp,
            tc.tile_pool(name="osb", bufs=2) as osb,
            tc.tile_pool(name="spsum", bufs=3, space="PSUM") as spsum,
            tc.tile_pool(name="ypsum", bufs=2, space="PSUM") as ypsum,
        ):
            # ---- input staging; order so slot 0's operands land first ----
            sb_qr = res.tile([P, N], mdt, tag="qr")
            sb_qi = res.tile([P, N], mdt, tag="qi")
            sb_k = res.tile([P, NJPAR * JB], mdt, tag="k")
            sb_m0 = res.tile([JB, 2, IBW], mdt, tag="m0")
            sb_m1 = res.tile([JB, 2, IBW], mdt, tag="m1")
            sb_var = res.tile([P, NJPAR * F], mdt, tag="var")
            sb_vai = res.tile([P, NJPAR * F], mdt, tag="vai")
            sb_vbr = res.tile([P, NJPAR * F], mdt, tag="vbr")
            sb_vbi = res.tile([P, NJPAR * F], mdt, tag="vbi")
            sb_mcr = res.tile([P, NSLOT * F], mdt, tag="mcr")
            sb_mci = res.tile([P, NSLOT * F], mdt, tag="mci")

            # All input DMAs on the Sync queue, ordered by first use (slot s
            # needs q chunk s, k chunk s//2, v chunk 1 at slot 4, mcorr at
            # slot 1's end).  Slot 0's operands go first.
            nc.sync.dma_start(out=sb_qr[:, 0:512], in_=qrT[:, 0:512])
            nc.sync.dma_start(out=sb_k[:, 0:128], in_=kp[:, 0:128])
            nc.sync.dma_start(out=sb_qi[:, 0:512], in_=qiT[:, 0:512])
            nc.sync.dma_start(out=sb_m0, in_=dmask[0])
            nc.sync.dma_start(out=sb_m1, in_=dmask[1])
            nc.sync.dma_start(out=sb_vbr[:, 0:512], in_=vbr[:, 0:512])
            nc.sync.dma_start(out=sb_vbi[:, 0:512], in_=vbi[:, 0:512])
            nc.sync.dma_start(out=sb_var[:, 0:512], in_=var_[:, 0:512])
            nc.sync.dma_start(out=sb_vai[:, 0:512], in_=vai[:, 0:512])
            nc.sync.dma_start(out=sb_k[:, 128:512], in_=kp[:, 128:512])
            nc.sync.dma_start(out=sb_qr[:, 512:1024], in_=qrT[:, 512:1024])
            nc.sync.dma_start(out=sb_qi[:, 512:1024], in_=qiT[:, 512:1024])
            nc.sync.dma_start(out=sb_mcr, in_=mcr[:])
            nc.sync.dma_start(out=sb_mci, in_=mci[:])
            nc.sync.dma_start(out=sb_k[:, 512:1024], in_=kp[:, 512:1024])
            for c in range(2, 8):
                sl = slice(c * 512, (c + 1) * 512)
                nc.sync.dma_start(out=sb_qr[:, sl], in_=qrT[:, sl])
                nc.sync.dma_start(out=sb_qi[:, sl], in_=qiT[:, sl])
                if c < 4:
                    nc.sync.dma_start(out=sb_k[:, sl], in_=kp[:, sl])
                if c == 4:
                    nc.sync.dma_start(out=sb_vbr[:, 512:1024],
                                      in_=vbr[:, 512:1024])
                    nc.sync.dma_start(out=sb_vbi[:, 512:1024],
                                      in_=vbi[:, 512:1024])
                    nc.sync.dma_start(out=sb_var[:, 512:1024],
                                      in_=var_[:, 512:1024])
                    nc.sync.dma_start(out=sb_vai[:, 512:1024],
                                      in_=vai[:, 512:1024])
            sb_masks = (sb_m0, sb_m1)

            # Warm the PE (HAM clock gate) with dummy matmuls while the first
            # DMAs are in flight, sized to span the DMA head so real matmuls
            # start at 2.4 GHz without being queued behind the warm-up.
            warm_sb = res.tile([P, F], mdt, tag="warm")
            nc.vector.memset(warm_sb[:], 0.0)
            warm_y = ypsum.tile([P, IBW], dt, tag="y")
            for _ in range(18):
                nc.tensor.matmul(warm_y[0:64, 0:64], warm_sb[:], warm_sb[:],
                                 start=True, stop=True)

            drain_ctr = 0  # full-tile relu drains: cycle V,S,S
            for s in range(NSLOT):
                cnt = 2 * s + 2
                isl = slice(s * IBW, (s + 1) * IBW)
                y = ypsum.tile([P, IBW], dt, tag="y")
                for p in range(cnt):
                    ksl = slice(p * JB, (p + 1) * JB)
                    vsl = slice(p * F, (p + 1) * F)
                    diag = p >= cnt - 2
                    k = p - (cnt - 2)
                    o = DIAG1_OFF if (diag and k == 1) else 0
                    qsl = slice(s * IBW + o, (s + 1) * IBW)
                    first = p == 0

                    s2 = spsum.tile([P, 2, IBW], dt, tag="s")
                    nc.tensor.matmul(s2[:, 0:1, o:], sb_k[:, ksl],
                                     sb_qr[:, qsl], start=True, stop=True)
                    nc.tensor.matmul(s2[:, 1:2, o:], sb_k[:, ksl],
                                     sb_qi[:, qsl], start=True, stop=True)

                    if not diag:
                        # full block: w = relu(s); 0.01*s is in mcorr
                        w = wp.tile([P, 2, IBW], mdt, tag="w")
                        if drain_ctr % 4 == 0:
                            nc.vector.tensor_scalar_max(w[:], s2[:], 0.0)
                        else:
                            nc.scalar.activation(w[:], s2[:], relu)
                        drain_ctr += 1
                        nc.tensor.matmul(y[0:64, :], sb_var[:, vsl],
                                         w[:, 0:1, :], start=first, stop=False)
                        nc.tensor.matmul(y[64:128, :], sb_vai[:, vsl],
                                         w[:, 1:2, :], start=first, stop=False)
                    else:
                        # diagonal block: u = mask*s (drains), then
                        # w2 = u + 99*relu(u) so that the single matmul pair
                        # vb @ w2 = 0.01*V'*u + 0.99*V'*relu(u) covers both
                        # leaky terms.
                        mk = sb_masks[k]
                        u = wp.tile([P, 2, IBW], mdt, tag="u")
                        nc.vector.tensor_tensor(out=u[:, :, o:], in0=s2[:, :, o:],
                                                in1=mk[:, :, o:], op=mul_op)
                        a = wp.tile([P, 2, IBW], mdt, tag="a")
                        nc.vector.tensor_scalar(out=a[:, :, o:], in0=u[:, :, o:],
                                                scalar1=0.0, scalar2=99.0,
                                                op0=max_op, op1=mul_op)
                        w2 = wp.tile([P, 2, IBW], mdt, tag="w2")
                        nc.vector.tensor_tensor(out=w2[:, :, o:], in0=a[:, :, o:],
                                                in1=u[:, :, o:], op=add_op)
                        last = s == 0 and p == cnt - 1
                        nc.tensor.matmul(y[0:64, o:], sb_vbr[:, vsl],
                                         w2[:, 0:1, o:], start=first, stop=last)
                        nc.tensor.matmul(y[64:128, o:], sb_vbi[:, vsl],
                                         w2[:, 1:2, o:], start=first, stop=last)
                # correction matmul: y += (0.01 * sum_full kp_J @ V'_J)^T @ q
                if s > 0:
                    msl = slice(s * F, (s + 1) * F)
                    nc.tensor.matmul(y[0:64, :], sb_mcr[:, msl],
                                     sb_qr[:, isl], start=False, stop=True)
                    nc.tensor.matmul(y[64:128, :], sb_mci[:, msl],
                                     sb_qi[:, isl], start=False, stop=True)
                # tail: accumulator to SBUF bf16 (alternate V/S), DMA out;
                # the last slot's DMA is split so its exposed tail halves
                y_sb = osb.tile([P, IBW], mdt, tag="ysb")
                if s % 2 == 0:
                    nc.vector.tensor_copy(y_sb[:], y[:])
                else:
                    nc.scalar.copy(y_sb[:], y[:])
                if s == NSLOT - 1:
                    nc.sync.dma_start(out=out[0:64, isl], in_=y_sb[0:64, :])
                    nc.gpsimd.dma_start(out=out[64:128, isl], in_=y_sb[64:128, :])
                else:
                    nc.sync.dma_start(out=out[:, isl], in_=y_sb[:])
    nc.compile()
    return nc


def _prep_inputs(Q, K, V, W_att, b_att):
    """Host-side re-layout: per-core in_maps for run_bass_kernel_spmd."""
    Q = np.asarray(Q, dtype=np.float32)
    K = np.asarray(K, dtype=np.float32)
    V = np.asarray(V, dtype=np.float32)
    W_att = np.asarray(W_att, dtype=np.float32)

    Qf = Q.reshape(B, N, P)          # [b, i, f*2+c]
    Kf = K.reshape(B, N, P)
    Vpr = SCALE * (V[..., 0] @ W_att.T)   # [B, N, F]
    Vpi = SCALE * (V[..., 1] @ W_att.T)

    # causal masks for a slot's last two parity j-blocks, per core parity h:
    # diagonal sub-block d = 2k+h of the slot's group of 4; duplicated along
    # a component axis -> [2, JB, 2, IBW]
    jj = np.arange(JB)[:, None]
    ii = np.arange(IBW)[None, :]
    masks = {}
    for h in (0, 1):
        mk = np.stack([(ii >= jj + JB * (2 * k + h)).astype(np.float32)
                       for k in range(2)])            # [2, JB, IBW]
        masks[h] = np.repeat(mk[:, :, None, :], 2, axis=2)  # [2, JB, 2, IBW]

    if MM_BF16:
        import ml_dtypes
        cvt = lambda a: np.ascontiguousarray(a).astype(ml_dtypes.bfloat16)
    else:
        cvt = lambda a: np.ascontiguousarray(a, dtype=np.float32)

    in_maps = []
    for c in range(NCORES):
        b, h = divmod(c, 2)
        Qmodr = Qf[b].copy()
        Qmodr[:, 1::2] *= -1.0
        Qmodi = np.empty_like(Qf[b])
        Qmodi[:, 0::2] = Qf[b][:, 1::2]
        Qmodi[:, 1::2] = Qf[b][:, 0::2]
        # parity-packed K: [P, NJPAR*JB], position pp holds block J = 2*pp+h
        kp3 = Kf[b].reshape(N // JB, JB, P)[h::2]          # [16, j, p]
        kp = kp3.transpose(2, 0, 1).reshape(P, -1)         # [p, pp*JB+j]
        vr3 = Vpr[b].reshape(N // JB, JB, F)[h::2]         # [16, j, f]
        vi3 = Vpi[b].reshape(N // JB, JB, F)[h::2]
        vpr = vr3.transpose(1, 0, 2).reshape(JB, -1)       # [j, pp*F+f]
        vpi = vi3.transpose(1, 0, 2).reshape(JB, -1)
        # per-slot correction: 0.01 * sum over FULL blocks (pos < cnt-2 = 2s)
        prod_r = np.einsum('bjp,bjf->bpf', kp3, vr3)       # [16, p, f]
        prod_i = np.einsum('bjp,bjf->bpf', kp3, vi3)
        pre_r = np.concatenate(
            [np.zeros((1, P, F), np.float32), np.cumsum(prod_r, axis=0)])
        pre_i = np.concatenate(
            [np.zeros((1, P, F), np.float32), np.cumsum(prod_i, axis=0)])
        mcr = np.concatenate([NEG * pre_r[2 * s] for s in range(NSLOT)], axis=1)
        mci = np.concatenate([NEG * pre_i[2 * s] for s in range(NSLOT)], axis=1)
        in_maps.append({
            "qrT": cvt(Qmodr.T),
            "qiT": cvt(Qmodi.T),
            "kp": cvt(kp),
            "var": cvt((1.0 - NEG) * vpr),
            "vai": cvt((1.0 - NEG) * vpi),
            "vbr": cvt(NEG * vpr),
            "vbi": cvt(NEG * vpi),
            "mcr": cvt(mcr),
            "mci": cvt(mci),
            "dmask": cvt(masks[h]),
        })
    return in_maps


def _gather(results, b_att):
    b_att = np.asarray(b_att, dtype=np.float32)
    out = np.empty((B, N, F, 2), dtype=np.float32)
    for b in range(B):
        y = (np.asarray(results[2 * b]["out"], dtype=np.float32)
             + np.asarray(results[2 * b + 1]["out"], dtype=np.float32))
        out[b, :, :, 0] = y[0:64].T + b_att[None, :]
        out[b, :, :, 1] = y[64:128].T + b_att[None, :]
    return out


def kernel(Q, K, V, W_att, b_att):
    if "nc" not in _CACHE:
        _CACHE["nc"] = _build_nc()
    nc = _CACHE["nc"]
    in_maps = _prep_inputs(Q, K, V, W_att, b_att)
    res = run_bass_kernel_spmd(nc, in_maps, core_ids=list(range(NCORES)))
    return _gather(res.results, b_att)
